# revision 32
# baseline (speedup 1.0000x reference)
"""Trainium2 Bass kernel for nn_ActivityHead (retrieval_knn).

Pipeline (per the reference):
  1. per-(batch, motion-token) top-3 nearest scene points by 2-D location
  2. fused = motion_feat + ALPHA * mean of the 3 gathered scene features
  3. x = fused.mean(tokens); MLP gelu(x@Wp+bp)@Wc+bc -> logits
  4. label-smoothed cross-entropy -> scalar loss

Key algebraic restructuring for the hardware:
  - ranking key: argmin dist <=> argmax (2*m.s - |s|^2), computed as one
    K=3 PE matmul (lhsT=[2mx;2my;1], rhs=[sx;sy;-|s|^2]) -> PSUM [128,1024]
  - top-3 via nc.vector.max (top-8 per partition) -> threshold at 3rd
    -> sel mask (one tensor_scalar), no gather at all
  - gather+means collapse into a weighted sum of scene rows:
    counts via sel^T @ const ones-matmuls, then a matvec streaming
    scene_feat once through the PE (float32r)
  - sharding: batch (32 -> 4 per core); MLP sharded over HID (2048 ->
    256 per core); AllGather x [4,1024]->[32,1024], partial logits
    AllReduce (+bc/8 per core); loss computed redundantly on every core
"""

import os
import sys
import types

sys.path.insert(0, "/opt/trn_rl_repo")
import numpy as np

N_CORES = 8
B, Nm, No, D = 32, 256, 1024, 1024
HID, C, K = 2048, 200, 3
EPS = 0.001
ALPHA = 0.3
BLOC = B // N_CORES          # 4 batches per core
HLOC = HID // N_CORES        # 256 hidden per core
CSEL = ALPHA / (Nm * K)      # weight per selected scene row
CMOT = 1.0 / Nm              # weight per motion row

LAST_EXEC_NS = None
LAST_RESULTS = None


def _install_trace_shims():
    """Make trace=True work under axon (agent image lacks antenv.axon_hooks)."""
    try:
        import antenv  # noqa: F401

        if "antenv.axon_hooks" not in sys.modules:
            hooks = types.ModuleType("antenv.axon_hooks")
            hooks._hook = None
            hooks.set_axon_ntff_profile_hook = lambda h: setattr(hooks, "_hook", h)
            hooks.get_axon_ntff_profile_hook = lambda: hooks._hook
            sys.modules["antenv.axon_hooks"] = hooks
            try:
                from trn_agent_boot.trn_boot import _ntff_profile_via_ctypes

                hooks.set_axon_ntff_profile_hook(
                    _ntff_profile_via_ctypes("/opt/axon/libaxon_pjrt.so")
                )
            except Exception:
                pass
        import concourse.bass_utils as bu

        if not getattr(bu, "_upload_patched", False):
            bu.upload_artifacts = lambda tmpdir: "local://" + tmpdir
            bu._upload_patched = True
    except Exception:
        pass


def build_graph():
    import concourse.bacc as bacc
    import concourse.bass as bass
    import concourse.mybir as mybir
    import concourse.tile as tile
    from concourse import masks

    f32 = mybir.dt.float32
    bf16 = mybir.dt.bfloat16
    Alu = mybir.AluOpType
    Act = mybir.ActivationFunctionType

    nc = bacc.Bacc("TRN2", target_bir_lowering=False, debug=False,
                   num_devices=N_CORES)

    scene_feat = nc.declare_dram_parameter("scene_feat", [BLOC, No, D], f32, isOutput=False)
    motion_feat = nc.declare_dram_parameter("motion_feat", [BLOC, Nm, D], f32, isOutput=False)
    scene_loc = nc.declare_dram_parameter("scene_loc", [BLOC, No, 2], f32, isOutput=False)
    # coordinate rows pre-transposed on host (input marshalling):
    # mloc_t = [mx; my] over (b n), sloc_t = [sx; sy] over (b o)
    mloc_t = nc.declare_dram_parameter("mloc_t", [2, BLOC * Nm], f32, isOutput=False)
    sloc_t = nc.declare_dram_parameter("sloc_t", [2, BLOC * No], f32, isOutput=False)
    wp = nc.declare_dram_parameter("wp", [D, HLOC], f32, isOutput=False)
    bp = nc.declare_dram_parameter("bp", [1, HLOC], f32, isOutput=False)
    wc = nc.declare_dram_parameter("wc", [HLOC, C], f32, isOutput=False)
    bc8 = nc.declare_dram_parameter("bc8", [1, C], f32, isOutput=False)
    # onehot rows are in device batch order (lb*8 + core); host unpermutes
    onehot = nc.declare_dram_parameter("onehot", [B, C], f32, isOutput=False)
    out_logit = nc.declare_dram_parameter("logit", [B, C], f32, isOutput=True)
    out_loss = nc.declare_dram_parameter("loss", [1, 1], f32, isOutput=True)

    rg = [list(range(N_CORES))]

    with tile.TileContext(nc) as tc:
        with (
            tc.tile_pool(name="const", bufs=1) as constp,
            tc.tile_pool(name="prep", bufs=1) as prep,
            tc.tile_pool(name="wts", bufs=1) as wts,
            tc.tile_pool(name="work", bufs=2) as work,
            tc.tile_pool(name="mot", bufs=2) as motp,
            tc.tile_pool(name="scn", bufs=2) as scnp,
            tc.tile_pool(name="scnb", bufs=2) as scnbp,
            tc.tile_pool(name="late", bufs=1) as late,
            tc.tile_pool(name="pk", bufs=2, space=bass.MemorySpace.PSUM) as pk,
            tc.tile_pool(name="pw", bufs=2, space=bass.MemorySpace.PSUM) as pw,
            tc.tile_pool(name="px", bufs=1, space=bass.MemorySpace.PSUM) as px,
            tc.tile_pool(name="dram", bufs=1, space="DRAM") as dram,
        ):
            # ---- constants -------------------------------------------------
            cvec = constp.tile([128, 1], bf16, tag="cvec")
            nc.vector.memset(cvec[:], 1.0)
            # 1/256 is a power of two: exact in bf16
            onesv = constp.tile([128, 1], bf16, tag="onesv")
            nc.vector.memset(onesv[:], CMOT)
            ones1 = constp.tile([1, B], f32, tag="ones1")
            nc.vector.memset(ones1[:], 1.0)
            ones32 = constp.tile([B, 1], f32, tag="ones32")
            nc.vector.memset(ones32[:], 1.0 / B)
            ident = constp.tile([128, 128], f32, tag="ident")
            masks.make_identity(nc, ident[:])

            # ---- location prep --------------------------------------------
            # mp: [2*mx; 2*my; 1] laid out [3, BLOC*Nm]
            mp = prep.tile([3, BLOC * Nm], f32, tag="mp")
            nc.vector.memset(mp[:], 1.0)  # row 2 stays 1.0
            nc.sync.dma_start(mp[0:2, :], mloc_t.ap())
            nc.vector.tensor_scalar(mp[0:2, :], mp[0:2, :], 2.0, None, Alu.mult)

            # sp: [sx; sy; -(sx^2+sy^2)] laid out [3, BLOC*No]
            sp = prep.tile([3, BLOC * No], f32, tag="sp")
            nc.sync.dma_start(sp[0:2, :], sloc_t.ap())
            # -(sx^2+sy^2) per batch, computed partition-aligned [BLOC, No]
            sraw = prep.tile([BLOC, 2 * No], f32, tag="sraw")
            nc.sync.dma_start(sraw[:], scene_loc.ap().rearrange("b o c -> b (o c)"))
            sqx = prep.tile([BLOC, No], f32, tag="sqx")
            sqy = prep.tile([BLOC, No], f32, tag="sqy")
            nc.vector.tensor_tensor(
                sqx[:], sraw[:, 0:2 * No:2], sraw[:, 0:2 * No:2], Alu.mult
            )
            nc.vector.tensor_tensor(
                sqy[:], sraw[:, 1:2 * No:2], sraw[:, 1:2 * No:2], Alu.mult
            )
            negs2 = prep.tile([BLOC, No], f32, tag="negs2")
            nc.vector.scalar_tensor_tensor(
                negs2[:], sqx[:], -1.0, sqy[:], Alu.mult, Alu.subtract
            )
            for b in range(BLOC):
                # cross-partition move (row b -> row 2): must be DMA
                nc.sync.dma_start(sp[2:3, b * No:(b + 1) * No], negs2[b:b + 1, :])

            # ---- weights / misc loads (single DMA each) -------------------
            wp_sb = wts.tile([128, 8, HLOC], f32, tag="wp")
            nc.sync.dma_start(
                wp_sb[:], wp.ap().rearrange("(k p) h -> p k h", p=128)
            )
            wc_sb = wts.tile([128, 2, C], f32, tag="wc")
            nc.sync.dma_start(
                wc_sb[:], wc.ap().rearrange("(k p) c -> p k c", p=128)
            )
            bp_sb = wts.tile([1, HLOC], f32, tag="bp")
            nc.sync.dma_start(bp_sb[:], bp.ap())
            bc8_sb = wts.tile([1, C], f32, tag="bc8")
            nc.sync.dma_start(bc8_sb[:], bc8.ap())
            oh_sb = wts.tile([B, C], f32, tag="oh")
            nc.sync.dma_start(oh_sb[:], onehot.ap())
            # loss coefficients, ready long before the tail:
            # coef = -(1-eps)*onehot - eps/C
            coef = wts.tile([B, C], f32, tag="coef")
            nc.vector.tensor_scalar(
                coef[:], oh_sb[:], -(1.0 - EPS), -EPS / C, Alu.mult, Alu.add
            )
            # warm the ACT function tables used in the tail
            warm_in = constp.tile([1, 8], f32, tag="warm_in")
            nc.vector.memset(warm_in[:], 0.5)
            warm = constp.tile([1, 8], f32, tag="warm")
            nc.scalar.activation(warm[:], warm_in[:], Act.Gelu)
            nc.scalar.activation(warm[:], warm_in[:], Act.Exp)
            nc.scalar.activation(warm[:], warm_in[:], Act.Ln)

            # ---- phase B: top-3 selection masks -> scene-row weights ------
            wcol_all = prep.tile([128, BLOC * 8], bf16, tag="wcol_all")
            for b in range(BLOC):
                wcol_ps = pw.tile([128, 8], f32, tag="wcol")
                sels = []
                for j in range(2):
                    key_ps = pk.tile([128, No], f32, tag="key")
                    lhsT = mp[:, b * Nm + j * 128: b * Nm + (j + 1) * 128]
                    for dh in range(2):
                        nc.tensor.matmul(
                            key_ps[:, dh * 512:(dh + 1) * 512],
                            lhsT,
                            sp[:, b * No + dh * 512: b * No + (dh + 1) * 512],
                        )
                    ksb = work.tile([128, No], f32, tag="ksb")
                    nc.scalar.copy(ksb[:], key_ps[:])
                    m8 = work.tile([128, 8], f32, tag="m8")
                    nc.vector.max(m8[:], ksb[:])
                    sel = work.tile([128, No], bf16, tag="sel")
                    nc.vector.tensor_scalar(
                        sel[:], ksb[:], m8[:, 2:3], None, Alu.is_ge
                    )
                    sels.append(sel)
                for c in range(8):
                    for j in range(2):
                        nc.tensor.matmul(
                            wcol_ps[:, c:c + 1],
                            sels[j][:, c * 128:(c + 1) * 128],
                            cvec[:],
                            start=(c == 0 and j == 0),
                            stop=(c == 7 and j == 1),
                        )
                # counts * CSEL, rounded to bf16 for the scene matvec lhsT
                nc.scalar.activation(
                    wcol_all[:, b * 8:(b + 1) * 8], wcol_ps[:], Act.Copy,
                    scale=CSEL,
                )

            # ---- phase C: x = mean(motion) + weighted scene rows ----------
            # One DMA per batch for motion and scene; per-batch AllGather of
            # the finished x row, overlapped with later batches' compute.
            # Device batch order after gather: lb*8 + core (host unpermutes).
            xg = late.tile([B, D], f32, tag="xg")
            ccouts = []
            for b in range(BLOC):
                xacc = px.tile([1, D], f32, tag="xacc")
                mt = motp.tile([128, 2, D], f32, tag="mt")
                nc.sync.dma_start(
                    mt[:],
                    motion_feat.ap()[b].rearrange("(j p) d -> p j d", p=128),
                )
                mtb = motp.tile([128, 2, D], bf16, tag="mtb")
                nc.vector.tensor_copy(mtb[:], mt[:])
                st = scnp.tile([128, 8, D], f32, tag="st")
                nc.sync.dma_start(
                    st[:],
                    scene_feat.ap()[b].rearrange("(c p) d -> p c d", p=128),
                )
                stb = scnbp.tile([128, 8, D], bf16, tag="stb")
                nc.scalar.copy(stb[:], st[:])
                for j in range(2):
                    for dh in range(2):
                        nc.tensor.matmul(
                            xacc[:, dh * 512:(dh + 1) * 512],
                            onesv[:],
                            mtb[:, j, dh * 512:(dh + 1) * 512],
                            start=(j == 0),
                            stop=False,
                        )
                for c in range(8):
                    for dh in range(2):
                        nc.tensor.matmul(
                            xacc[:, dh * 512:(dh + 1) * 512],
                            wcol_all[:, b * 8 + c: b * 8 + c + 1],
                            stb[:, c, dh * 512:(dh + 1) * 512],
                            start=False,
                            stop=(c == 7),
                        )
                xtmp = work.tile([1, D], f32, tag="xtmp")
                nc.vector.tensor_copy(xtmp[:], xacc[:])
                ccin = dram.tile([1, D], f32, tag=f"ccin{b}")
                ccout = dram.tile([N_CORES, D], f32, tag=f"ccout{b}")
                nc.sync.dma_start(ccin[:], xtmp[:])
                nc.gpsimd.collective_compute(
                    "AllGather", mybir.AluOpType.bypass,
                    replica_groups=rg, ins=[ccin.opt()], outs=[ccout.opt()],
                )
                ccouts.append(ccout)

            # ---- phase D: assemble gathered x -----------------------------
            for b in range(BLOC):
                nc.sync.dma_start(
                    xg[b * N_CORES:(b + 1) * N_CORES, :], ccouts[b][:]
                )

            # ---- phase E: sharded MLP -------------------------------------
            xT = late.tile([128, 8 * B], f32, tag="xT")
            for k in range(8):
                tp = pk.tile([128, B], f32, tag="key")
                nc.tensor.transpose(
                    tp[:], xg[:, k * 128:(k + 1) * 128], ident[0:B, 0:B]
                )
                nc.vector.tensor_copy(xT[:, k * B:(k + 1) * B], tp[:])
            h_ps = pk.tile([B, HLOC], f32, tag="key")
            for k in range(8):
                nc.tensor.matmul(
                    h_ps[:],
                    xT[:, k * B:(k + 1) * B],
                    wp_sb[:, k, :],
                    start=(k == 0),
                    stop=False,
                )
            nc.tensor.matmul(h_ps[:], ones1[:], bp_sb[:], start=False, stop=True)
            h_sb = late.tile([B, HLOC], f32, tag="h")
            nc.scalar.activation(h_sb[:], h_ps[:], Act.Gelu)
            hT = late.tile([128, 2 * B], f32, tag="hT")
            for k in range(2):
                tp = pk.tile([128, B], f32, tag="key")
                nc.tensor.transpose(
                    tp[:], h_sb[:, k * 128:(k + 1) * 128], ident[0:B, 0:B]
                )
                nc.vector.tensor_copy(hT[:, k * B:(k + 1) * B], tp[:])
            lg_ps = pk.tile([B, C], f32, tag="key")
            for k in range(2):
                nc.tensor.matmul(
                    lg_ps[:],
                    hT[:, k * B:(k + 1) * B],
                    wc_sb[:, k, :],
                    start=(k == 0),
                    stop=False,
                )
            nc.tensor.matmul(lg_ps[:], ones1[:], bc8_sb[:], start=False, stop=True)
            lgp = late.tile([B, C], f32, tag="lgp")
            nc.vector.tensor_copy(lgp[:], lg_ps[:])

            cc2in = dram.tile([B, C], f32, tag="cc2in")
            cc2out = dram.tile([B, C], f32, tag="cc2out")
            nc.sync.dma_start(cc2in[:], lgp[:])
            nc.gpsimd.collective_compute(
                "AllReduce", mybir.AluOpType.add,
                replica_groups=rg, ins=[cc2in.opt()], outs=[cc2out.opt()],
            )
            lg = late.tile([B, C], f32, tag="lg")
            nc.sync.dma_start(lg[:], cc2out[:])
            nc.sync.dma_start(out_logit.ap(), lg[:])

            # ---- phase F: label-smoothed CE loss --------------------------
            rmax = late.tile([B, 1], f32, tag="rmax")
            nc.vector.reduce_max(rmax[:], lg[:], axis=mybir.AxisListType.X)
            rmaxn = late.tile([B, 1], f32, tag="rmaxn")
            nc.vector.tensor_scalar(rmaxn[:], rmax[:], -1.0, None, Alu.mult)
            esum = late.tile([B, 1], f32, tag="esum")
            etile = late.tile([B, C], f32, tag="etile")
            nc.scalar.activation(
                etile[:], lg[:], Act.Exp, bias=rmaxn[:, 0:1], accum_out=esum[:]
            )
            lns = late.tile([B, 1], f32, tag="lns")
            nc.scalar.activation(lns[:], esum[:], Act.Ln)
            lse = late.tile([B, 1], f32, tag="lse")
            nc.vector.tensor_tensor(lse[:], lns[:], rmax[:], Alu.add)
            # prod = (logit - lse) * coef, fused
            prod = late.tile([B, C], f32, tag="prod")
            nc.vector.scalar_tensor_tensor(
                prod[:], lg[:], lse[:, 0:1], coef[:], Alu.subtract, Alu.mult
            )
            lossb = late.tile([B, 1], f32, tag="lossb")
            nc.vector.reduce_sum(lossb[:], prod[:], axis=mybir.AxisListType.X)
            loss_ps = pw.tile([1, 1], f32, tag="wcol")
            nc.tensor.matmul(loss_ps[:], lossb[:], ones32[:])
            loss_sb = late.tile([1, 1], f32, tag="loss_sb")
            nc.vector.tensor_copy(loss_sb[:], loss_ps[:])
            nc.sync.dma_start(out_loss.ap(), loss_sb[:])

    nc.compile()
    return nc


_GRAPH = None


def kernel(scene_feat, motion_feat, scene_loc, motion_loc, label, Wp, bp, Wc, bc):
    global _GRAPH, LAST_EXEC_NS, LAST_RESULTS
    _install_trace_shims()
    from concourse.bass_utils import run_bass_kernel_spmd

    scene_feat = np.ascontiguousarray(np.asarray(scene_feat, dtype=np.float32))
    motion_feat = np.ascontiguousarray(np.asarray(motion_feat, dtype=np.float32))
    scene_loc = np.ascontiguousarray(np.asarray(scene_loc, dtype=np.float32))
    motion_loc = np.ascontiguousarray(np.asarray(motion_loc, dtype=np.float32))
    label_np = np.asarray(label).astype(np.int64)
    Wp = np.asarray(Wp, dtype=np.float32)
    bp_np = np.asarray(bp, dtype=np.float32)
    Wc = np.asarray(Wc, dtype=np.float32)
    bc_np = np.asarray(bc, dtype=np.float32)

    onehot = np.zeros((B, C), dtype=np.float32)
    onehot[np.arange(B), label_np] = 1.0
    # device batch order is lb*8 + core; row i holds global batch perm[i]
    perm = np.array([(i % N_CORES) * BLOC + i // N_CORES for i in range(B)])
    onehot_dev = np.ascontiguousarray(onehot[perm])

    if _GRAPH is None:
        _GRAPH = build_graph()
    nc = _GRAPH

    in_maps = []
    for i in range(N_CORES):
        bs = slice(i * BLOC, (i + 1) * BLOC)
        hs = slice(i * HLOC, (i + 1) * HLOC)
        # pre-transposed coordinate rows (layout marshalling only)
        ml = motion_loc[bs]  # [BLOC, Nm, 2]
        sl = scene_loc[bs]   # [BLOC, No, 2]
        mloc_t = np.ascontiguousarray(
            ml.transpose(2, 0, 1).reshape(2, BLOC * Nm)
        )
        sloc_t = np.ascontiguousarray(
            sl.transpose(2, 0, 1).reshape(2, BLOC * No)
        )
        in_maps.append({
            "scene_feat": scene_feat[bs],
            "motion_feat": motion_feat[bs],
            "scene_loc": sl,
            "mloc_t": mloc_t,
            "sloc_t": sloc_t,
            "wp": np.ascontiguousarray(Wp[:, hs]),
            "bp": np.ascontiguousarray(bp_np[hs]).reshape(1, HLOC),
            "wc": np.ascontiguousarray(Wc[hs, :]),
            "bc8": (bc_np / N_CORES).reshape(1, C),
            "onehot": onehot_dev,
        })

    trace = bool(os.environ.get("BASS_TRACE"))
    res = run_bass_kernel_spmd(
        nc, in_maps, core_ids=list(range(N_CORES)), trace=trace
    )
    LAST_EXEC_NS = res.exec_time_ns
    LAST_RESULTS = res
    logit_dev = np.asarray(res.results[0]["logit"], dtype=np.float32)
    logit = np.empty_like(logit_dev)
    logit[perm] = logit_dev
    loss = np.float32(np.asarray(res.results[0]["loss"]).reshape(-1)[0])
    return logit, loss


# revision 35
# speedup vs baseline: 1.2420x; 1.2420x over previous
"""Trainium2 Bass kernel for nn_ActivityHead (retrieval_knn).

Pipeline (per the reference):
  1. per-(batch, motion-token) top-3 nearest scene points by 2-D location
  2. fused = motion_feat + ALPHA * mean of the 3 gathered scene features
  3. x = fused.mean(tokens); MLP gelu(x@Wp+bp)@Wc+bc -> logits
  4. label-smoothed cross-entropy -> scalar loss

Key algebraic restructuring for the hardware:
  - ranking key: argmin dist <=> argmax (2*m.s - |s|^2), computed as one
    K=3 PE matmul (lhsT=[2mx;2my;1], rhs=[sx;sy;-|s|^2]) -> PSUM [128,1024]
  - top-3 via nc.vector.max (top-8 per partition) -> threshold at 3rd
    -> sel mask (one tensor_scalar), no gather at all
  - gather+means collapse into a weighted sum of scene rows:
    counts via sel^T @ const ones-matmuls, then a matvec streaming
    scene_feat once through the PE (float32r)
  - sharding: batch (32 -> 4 per core); MLP sharded over HID (2048 ->
    256 per core); AllGather x [4,1024]->[32,1024], partial logits
    AllReduce (+bc/8 per core); loss computed redundantly on every core
"""

import os
import sys
import types

sys.path.insert(0, "/opt/trn_rl_repo")
import numpy as np

N_CORES = 8
B, Nm, No, D = 32, 256, 1024, 1024
HID, C, K = 2048, 200, 3
EPS = 0.001
ALPHA = 0.3
BLOC = B // N_CORES          # 4 batches per core
HLOC = HID // N_CORES        # 256 hidden per core
CSEL = ALPHA / (Nm * K)      # weight per selected scene row
CMOT = 1.0 / Nm              # weight per motion row

LAST_EXEC_NS = None
LAST_RESULTS = None


def _install_trace_shims():
    """Make trace=True work under axon (agent image lacks antenv.axon_hooks)."""
    try:
        import antenv  # noqa: F401

        if "antenv.axon_hooks" not in sys.modules:
            hooks = types.ModuleType("antenv.axon_hooks")
            hooks._hook = None
            hooks.set_axon_ntff_profile_hook = lambda h: setattr(hooks, "_hook", h)
            hooks.get_axon_ntff_profile_hook = lambda: hooks._hook
            sys.modules["antenv.axon_hooks"] = hooks
            try:
                from trn_agent_boot.trn_boot import _ntff_profile_via_ctypes

                hooks.set_axon_ntff_profile_hook(
                    _ntff_profile_via_ctypes("/opt/axon/libaxon_pjrt.so")
                )
            except Exception:
                pass
        import concourse.bass_utils as bu

        if not getattr(bu, "_upload_patched", False):
            bu.upload_artifacts = lambda tmpdir: "local://" + tmpdir
            bu._upload_patched = True
    except Exception:
        pass


def build_graph():
    import concourse.bacc as bacc
    import concourse.bass as bass
    import concourse.mybir as mybir
    import concourse.tile as tile
    from concourse import masks

    f32 = mybir.dt.float32
    bf16 = mybir.dt.bfloat16
    Alu = mybir.AluOpType
    Act = mybir.ActivationFunctionType

    nc = bacc.Bacc("TRN2", target_bir_lowering=False, debug=False,
                   num_devices=N_CORES)

    scene_feat = nc.declare_dram_parameter("scene_feat", [BLOC, No, D], f32, isOutput=False)
    motion_feat = nc.declare_dram_parameter("motion_feat", [BLOC, Nm, D], f32, isOutput=False)
    scene_loc = nc.declare_dram_parameter("scene_loc", [BLOC, No, 2], f32, isOutput=False)
    # coordinate rows pre-transposed on host (input marshalling):
    # mloc_t = [mx; my] over (b n), sloc_t = [sx; sy] over (b o)
    mloc_t = nc.declare_dram_parameter("mloc_t", [2, BLOC * Nm], f32, isOutput=False)
    sloc_t = nc.declare_dram_parameter("sloc_t", [2, BLOC * No], f32, isOutput=False)
    wp = nc.declare_dram_parameter("wp", [D, HLOC], f32, isOutput=False)
    bp = nc.declare_dram_parameter("bp", [1, HLOC], f32, isOutput=False)
    wc = nc.declare_dram_parameter("wc", [HLOC, C], f32, isOutput=False)
    bc8 = nc.declare_dram_parameter("bc8", [1, C], f32, isOutput=False)
    # onehot rows are in device batch order (lb*8 + core); host unpermutes
    onehot = nc.declare_dram_parameter("onehot", [B, C], f32, isOutput=False)
    out_logit = nc.declare_dram_parameter("logit", [B, C], f32, isOutput=True)
    out_loss = nc.declare_dram_parameter("loss", [1, 1], f32, isOutput=True)

    rg = [list(range(N_CORES))]

    with tile.TileContext(nc) as tc:
        with (
            tc.tile_pool(name="const", bufs=1) as constp,
            tc.tile_pool(name="prep", bufs=1) as prep,
            tc.tile_pool(name="wts", bufs=1) as wts,
            tc.tile_pool(name="work", bufs=2) as work,
            tc.tile_pool(name="mot", bufs=2) as motp,
            tc.tile_pool(name="scn", bufs=2) as scnp,
            tc.tile_pool(name="scnb", bufs=2) as scnbp,
            tc.tile_pool(name="late", bufs=1) as late,
            tc.tile_pool(name="pk", bufs=2, space=bass.MemorySpace.PSUM) as pk,
            tc.tile_pool(name="pw", bufs=2, space=bass.MemorySpace.PSUM) as pw,
            tc.tile_pool(name="px", bufs=1, space=bass.MemorySpace.PSUM) as px,
            tc.tile_pool(name="dram", bufs=1, space="DRAM") as dram,
        ):
            # ---- constants -------------------------------------------------
            cvec = constp.tile([128, 1], bf16, tag="cvec")
            nc.vector.memset(cvec[:], 1.0)
            # 1/256 is a power of two: exact in bf16
            onesv = constp.tile([128, 1], bf16, tag="onesv")
            nc.vector.memset(onesv[:], CMOT)
            ones1 = constp.tile([1, B], f32, tag="ones1")
            nc.vector.memset(ones1[:], 1.0)
            ones32 = constp.tile([B, 1], f32, tag="ones32")
            nc.vector.memset(ones32[:], 1.0 / B)
            ident = constp.tile([128, 128], f32, tag="ident")
            masks.make_identity(nc, ident[:])

            # ---- location prep --------------------------------------------
            # mp: [2*mx; 2*my; 1] laid out [3, BLOC*Nm]
            mp = prep.tile([3, BLOC * Nm], f32, tag="mp")
            nc.vector.memset(mp[:], 1.0)  # row 2 stays 1.0
            nc.sync.dma_start(mp[0:2, :], mloc_t.ap())
            nc.vector.tensor_scalar(mp[0:2, :], mp[0:2, :], 2.0, None, Alu.mult)

            # sp: [sx; sy; -(sx^2+sy^2)] laid out [3, BLOC*No]
            sp = prep.tile([3, BLOC * No], f32, tag="sp")
            nc.sync.dma_start(sp[0:2, :], sloc_t.ap())
            # -(sx^2+sy^2) per batch, computed partition-aligned [BLOC, No]
            sraw = prep.tile([BLOC, 2 * No], f32, tag="sraw")
            nc.sync.dma_start(sraw[:], scene_loc.ap().rearrange("b o c -> b (o c)"))
            sqx = prep.tile([BLOC, No], f32, tag="sqx")
            sqy = prep.tile([BLOC, No], f32, tag="sqy")
            nc.vector.tensor_tensor(
                sqx[:], sraw[:, 0:2 * No:2], sraw[:, 0:2 * No:2], Alu.mult
            )
            nc.vector.tensor_tensor(
                sqy[:], sraw[:, 1:2 * No:2], sraw[:, 1:2 * No:2], Alu.mult
            )
            negs2 = prep.tile([BLOC, No], f32, tag="negs2")
            nc.vector.scalar_tensor_tensor(
                negs2[:], sqx[:], -1.0, sqy[:], Alu.mult, Alu.subtract
            )
            for b in range(BLOC):
                # cross-partition move (row b -> row 2): must be DMA
                nc.sync.dma_start(sp[2:3, b * No:(b + 1) * No], negs2[b:b + 1, :])

            # ---- weights / misc loads (single DMA each) -------------------
            wp_sb = wts.tile([128, 8, HLOC], f32, tag="wp")
            nc.sync.dma_start(
                wp_sb[:], wp.ap().rearrange("(k p) h -> p k h", p=128)
            )
            wc_sb = wts.tile([128, 2, C], f32, tag="wc")
            nc.sync.dma_start(
                wc_sb[:], wc.ap().rearrange("(k p) c -> p k c", p=128)
            )
            bp_sb = wts.tile([1, HLOC], f32, tag="bp")
            nc.sync.dma_start(bp_sb[:], bp.ap())
            bc8_sb = wts.tile([1, C], f32, tag="bc8")
            nc.sync.dma_start(bc8_sb[:], bc8.ap())
            oh_sb = wts.tile([B, C], f32, tag="oh")
            nc.sync.dma_start(oh_sb[:], onehot.ap())
            # loss coefficients, ready long before the tail:
            # coef = -(1-eps)*onehot - eps/C
            coef = wts.tile([B, C], f32, tag="coef")
            nc.vector.tensor_scalar(
                coef[:], oh_sb[:], -(1.0 - EPS), -EPS / C, Alu.mult, Alu.add
            )


            # ---- phase B: top-3 selection masks -> scene-row weights ------
            wcol_all = prep.tile([128, BLOC * 8], bf16, tag="wcol_all")
            for b in range(BLOC):
                wcol_ps = pw.tile([128, 8], f32, tag="wcol")
                sels = []
                for j in range(2):
                    key_ps = pk.tile([128, No], f32, tag="key")
                    lhsT = mp[:, b * Nm + j * 128: b * Nm + (j + 1) * 128]
                    for dh in range(2):
                        nc.tensor.matmul(
                            key_ps[:, dh * 512:(dh + 1) * 512],
                            lhsT,
                            sp[:, b * No + dh * 512: b * No + (dh + 1) * 512],
                        )
                    ksb = work.tile([128, No], f32, tag="ksb")
                    nc.scalar.copy(ksb[:], key_ps[:])
                    m8 = work.tile([128, 8], f32, tag="m8")
                    nc.vector.max(m8[:], ksb[:])
                    sel = work.tile([128, No], bf16, tag="sel")
                    nc.vector.tensor_scalar(
                        sel[:], ksb[:], m8[:, 2:3], None, Alu.is_ge
                    )
                    sels.append(sel)
                for c in range(8):
                    for j in range(2):
                        nc.tensor.matmul(
                            wcol_ps[:, c:c + 1],
                            sels[j][:, c * 128:(c + 1) * 128],
                            cvec[:],
                            start=(c == 0 and j == 0),
                            stop=(c == 7 and j == 1),
                        )
                # counts * CSEL, rounded to bf16 for the scene matvec lhsT
                nc.scalar.activation(
                    wcol_all[:, b * 8:(b + 1) * 8], wcol_ps[:], Act.Copy,
                    scale=CSEL,
                )

            # ---- phase C: x = mean(motion) + weighted scene rows ----------
            # One DMA per batch for motion and scene; per-batch AllGather of
            # the finished x row, overlapped with later batches' compute.
            # Device batch order after gather: lb*8 + core (host unpermutes).
            xg = late.tile([B, D], f32, tag="xg")
            ccouts = []
            for b in range(BLOC):
                xacc = px.tile([1, D], f32, tag="xacc")
                mt = motp.tile([128, 2 * D], f32, tag="mt")
                nc.sync.dma_start(
                    mt[:].rearrange("p (j d) -> p j d", j=2),
                    motion_feat.ap()[b].rearrange("(j p) d -> p j d", p=128),
                )
                mtb = motp.tile([128, 2 * D], bf16, tag="mtb")
                nc.vector.tensor_copy(mtb[:], mt[:])
                st = scnp.tile([128, 8 * D], f32, tag="st")
                nc.sync.dma_start(
                    st[:].rearrange("p (c d) -> p c d", c=8),
                    scene_feat.ap()[b].rearrange("(c p) d -> p c d", p=128),
                )
                stb = scnbp.tile([128, 8 * D], bf16, tag="stb")
                if b % 2 == 0:
                    nc.scalar.copy(stb[:], st[:])
                else:
                    nc.vector.tensor_copy(stb[:], st[:])
                for j in range(2):
                    for dh in range(2):
                        off = j * D + dh * 512
                        nc.tensor.matmul(
                            xacc[:, dh * 512:(dh + 1) * 512],
                            onesv[:],
                            mtb[:, off:off + 512],
                            start=(j == 0),
                            stop=False,
                        )
                for c in range(8):
                    for dh in range(2):
                        off = c * D + dh * 512
                        nc.tensor.matmul(
                            xacc[:, dh * 512:(dh + 1) * 512],
                            wcol_all[:, b * 8 + c: b * 8 + c + 1],
                            stb[:, off:off + 512],
                            start=False,
                            stop=(c == 7),
                        )
                xtmp = work.tile([1, D], f32, tag="xtmp")
                nc.vector.tensor_copy(xtmp[:], xacc[:])
                ccin = dram.tile([1, D], f32, tag=f"ccin{b}")
                ccout = dram.tile([N_CORES, D], f32, tag=f"ccout{b}")
                nc.gpsimd.dma_start(ccin[:], xtmp[:])
                nc.gpsimd.collective_compute(
                    "AllGather", mybir.AluOpType.bypass,
                    replica_groups=rg, ins=[ccin.opt()], outs=[ccout.opt()],
                )
                ccouts.append(ccout)

            # ---- phase D: assemble gathered x -----------------------------
            for b in range(BLOC):
                nc.gpsimd.dma_start(
                    xg[b * N_CORES:(b + 1) * N_CORES, :], ccouts[b][:]
                )

            # ---- phase E: sharded MLP -------------------------------------
            xT = late.tile([128, 8 * B], f32, tag="xT")
            for k in range(8):
                tp = pk.tile([128, B], f32, tag="key")
                nc.tensor.transpose(
                    tp[:], xg[:, k * 128:(k + 1) * 128], ident[0:B, 0:B]
                )
                nc.vector.tensor_copy(xT[:, k * B:(k + 1) * B], tp[:])
            h_ps = pk.tile([B, HLOC], f32, tag="key")
            for k in range(8):
                nc.tensor.matmul(
                    h_ps[:],
                    xT[:, k * B:(k + 1) * B],
                    wp_sb[:, k, :],
                    start=(k == 0),
                    stop=False,
                )
            nc.tensor.matmul(h_ps[:], ones1[:], bp_sb[:], start=False, stop=True)
            h_sb = late.tile([B, HLOC], f32, tag="h")
            nc.scalar.activation(h_sb[:], h_ps[:], Act.Gelu)
            hT = late.tile([128, 2 * B], f32, tag="hT")
            for k in range(2):
                tp = pk.tile([128, B], f32, tag="key")
                nc.tensor.transpose(
                    tp[:], h_sb[:, k * 128:(k + 1) * 128], ident[0:B, 0:B]
                )
                nc.vector.tensor_copy(hT[:, k * B:(k + 1) * B], tp[:])
            lg_ps = pk.tile([B, C], f32, tag="key")
            for k in range(2):
                nc.tensor.matmul(
                    lg_ps[:],
                    hT[:, k * B:(k + 1) * B],
                    wc_sb[:, k, :],
                    start=(k == 0),
                    stop=False,
                )
            nc.tensor.matmul(lg_ps[:], ones1[:], bc8_sb[:], start=False, stop=True)
            lgp = late.tile([B, C], f32, tag="lgp")
            nc.vector.tensor_copy(lgp[:], lg_ps[:])

            cc2in = dram.tile([B, C], f32, tag="cc2in")
            cc2out = dram.tile([B, C], f32, tag="cc2out")
            nc.gpsimd.dma_start(cc2in[:], lgp[:])
            nc.gpsimd.collective_compute(
                "AllReduce", mybir.AluOpType.add,
                replica_groups=rg, ins=[cc2in.opt()], outs=[cc2out.opt()],
            )
            lg = late.tile([B, C], f32, tag="lg")
            nc.gpsimd.dma_start(lg[:], cc2out[:])
            nc.gpsimd.dma_start(out_logit.ap(), lg[:])

            # ---- phase F: label-smoothed CE loss --------------------------
            rmax = late.tile([B, 1], f32, tag="rmax")
            nc.vector.reduce_max(rmax[:], lg[:], axis=mybir.AxisListType.X)
            rmaxn = late.tile([B, 1], f32, tag="rmaxn")
            nc.vector.tensor_scalar(rmaxn[:], rmax[:], -1.0, None, Alu.mult)
            esum = late.tile([B, 1], f32, tag="esum")
            etile = late.tile([B, C], f32, tag="etile")
            nc.scalar.activation(
                etile[:], lg[:], Act.Exp, bias=rmaxn[:, 0:1], accum_out=esum[:]
            )
            lns = late.tile([B, 1], f32, tag="lns")
            nc.scalar.activation(lns[:], esum[:], Act.Ln)
            lse = late.tile([B, 1], f32, tag="lse")
            nc.vector.tensor_tensor(lse[:], lns[:], rmax[:], Alu.add)
            # prod = (logit - lse) * coef, fused
            prod = late.tile([B, C], f32, tag="prod")
            nc.vector.scalar_tensor_tensor(
                prod[:], lg[:], lse[:, 0:1], coef[:], Alu.subtract, Alu.mult
            )
            lossb = late.tile([B, 1], f32, tag="lossb")
            nc.vector.reduce_sum(lossb[:], prod[:], axis=mybir.AxisListType.X)
            loss_ps = pw.tile([1, 1], f32, tag="wcol")
            nc.tensor.matmul(loss_ps[:], lossb[:], ones32[:])
            loss_sb = late.tile([1, 1], f32, tag="loss_sb")
            nc.vector.tensor_copy(loss_sb[:], loss_ps[:])
            nc.sync.dma_start(out_loss.ap(), loss_sb[:])

    nc.compile()
    return nc


_GRAPH = None


def kernel(scene_feat, motion_feat, scene_loc, motion_loc, label, Wp, bp, Wc, bc):
    global _GRAPH, LAST_EXEC_NS, LAST_RESULTS
    _install_trace_shims()
    from concourse.bass_utils import run_bass_kernel_spmd

    scene_feat = np.ascontiguousarray(np.asarray(scene_feat, dtype=np.float32))
    motion_feat = np.ascontiguousarray(np.asarray(motion_feat, dtype=np.float32))
    scene_loc = np.ascontiguousarray(np.asarray(scene_loc, dtype=np.float32))
    motion_loc = np.ascontiguousarray(np.asarray(motion_loc, dtype=np.float32))
    label_np = np.asarray(label).astype(np.int64)
    Wp = np.asarray(Wp, dtype=np.float32)
    bp_np = np.asarray(bp, dtype=np.float32)
    Wc = np.asarray(Wc, dtype=np.float32)
    bc_np = np.asarray(bc, dtype=np.float32)

    onehot = np.zeros((B, C), dtype=np.float32)
    onehot[np.arange(B), label_np] = 1.0
    # device batch order is lb*8 + core; row i holds global batch perm[i]
    perm = np.array([(i % N_CORES) * BLOC + i // N_CORES for i in range(B)])
    onehot_dev = np.ascontiguousarray(onehot[perm])

    if _GRAPH is None:
        _GRAPH = build_graph()
    nc = _GRAPH

    in_maps = []
    for i in range(N_CORES):
        bs = slice(i * BLOC, (i + 1) * BLOC)
        hs = slice(i * HLOC, (i + 1) * HLOC)
        # pre-transposed coordinate rows (layout marshalling only)
        ml = motion_loc[bs]  # [BLOC, Nm, 2]
        sl = scene_loc[bs]   # [BLOC, No, 2]
        mloc_t = np.ascontiguousarray(
            ml.transpose(2, 0, 1).reshape(2, BLOC * Nm)
        )
        sloc_t = np.ascontiguousarray(
            sl.transpose(2, 0, 1).reshape(2, BLOC * No)
        )
        in_maps.append({
            "scene_feat": scene_feat[bs],
            "motion_feat": motion_feat[bs],
            "scene_loc": sl,
            "mloc_t": mloc_t,
            "sloc_t": sloc_t,
            "wp": np.ascontiguousarray(Wp[:, hs]),
            "bp": np.ascontiguousarray(bp_np[hs]).reshape(1, HLOC),
            "wc": np.ascontiguousarray(Wc[hs, :]),
            "bc8": (bc_np / N_CORES).reshape(1, C),
            "onehot": onehot_dev,
        })

    trace = bool(os.environ.get("BASS_TRACE"))
    res = run_bass_kernel_spmd(
        nc, in_maps, core_ids=list(range(N_CORES)), trace=trace
    )
    LAST_EXEC_NS = res.exec_time_ns
    LAST_RESULTS = res
    logit_dev = np.asarray(res.results[0]["logit"], dtype=np.float32)
    logit = np.empty_like(logit_dev)
    logit[perm] = logit_dev
    loss = np.float32(np.asarray(res.results[0]["loss"]).reshape(-1)[0])
    return logit, loss
